# revision 15
# baseline (speedup 1.0000x reference)
"""Bass/Trainium2 kernel for nn_F_Loss_65446711656630.

Strategy (data-parallel over N, 8 cores):
  - Host: GLOBAL stable sort of all rows by class id, then slice 8192 rows
    per core and transpose to [512 features x 8192 rows] contiguous pieces
    (layout: partitions = features, free axis = rows).
  - Device (static kernel): stream 16 pieces of [128, 2048]; pieces are
    statically split between the two element-wise engines so both finish
    together:
      * DVE pieces ('V'): 4x bn_stats per piece (512-row groups). One DVE
        pass yields count/mean/M2 for even+odd lanes -> BOTH the h-sum and
        the h^2-sum for that granule. This is the key trick: the vector
        engine computes both statistics in a single pass over the data.
      * ACT pieces ('A'): Copy activation with accum_out (-> per-piece h
        sums) + Square activation with accum_out (-> per-piece h^2 sums).
      * One piece ('S') is split between the engines to fine-balance.
    Inputs ship as fp8 e4m3 when DT_IN='fp8' (final rel err ~2e-4, well
    inside the 2e-2 gate) halving HBM traffic to 4 MiB/core; fp16 fallback.
  - Host: per-class stats from pure-class granules (f64 accumulation)
    + direct numpy sums for the few granules spanning a class boundary;
    then the tiny O(C^2 D) pairwise betainc/top-k stage in f32 jax on CPU
    (mirroring the reference's numerics exactly).
"""

import numpy as np

C = 16
D = 512
N = 65536
NCORES = 8
ROWS = N // NCORES          # 8192 rows per core
P = 128                     # SBUF partitions
PIECE = 2048                # rows per DMA piece
G = 512                     # bn_stats group size (HW max)
GPP = PIECE // G            # 4 groups per piece
NBLK = D // P               # 4 feature blocks
NPIECE = ROWS // PIECE      # 4 row-pieces per core
NP_TOT = NBLK * NPIECE      # 16 pieces per core
XMIN, XMAX = 1e-37, 1.0 - 1e-5

# Engine assignment per piece: V=DVE bn_stats, A=ACT copy/square+accum,
# S=split (DVE groups 0-1, ACT rows 1024:2048).
ASSIGN = "VVAVVAVVAVAVVAVS"
assert len(ASSIGN) == NP_TOT

DT_IN = "fp8"               # "fp8" (e4m3) or "fp16"

_NC_CACHE = {}


def _np_in_dtype():
    if DT_IN == "fp8":
        import ml_dtypes
        return ml_dtypes.float8_e4m3fn
    return np.float16


def _build_nc():
    """Per-core SPMD program.

    Inputs:  "ht"    [16, 128, 2048] (piece i = b*4+p holds features
                                      b*128..+128 x rows p*2048..+2048)
    Outputs: "bn"    [128, 384] f32  (bn[f, i*24+g*6 : +6] = bn_stats of
                                      piece i group g: [ce,me,M2e,co,mo,M2o])
             "hacc"  [128, 16]  f32  (ACT pieces: per-piece h sums)
             "sqacc" [128, 16]  f32  (ACT pieces: per-piece h^2 sums)
    """
    import concourse.tile as tile
    from concourse import bacc, mybir

    f32 = mybir.dt.float32
    dt_in = mybir.dt.float8e4 if DT_IN == "fp8" else mybir.dt.float16
    AF = mybir.ActivationFunctionType

    nc = bacc.Bacc("TRN2", target_bir_lowering=False, debug=False,
                   num_devices=NCORES)
    ht = nc.declare_dram_parameter("ht", [NP_TOT, P, PIECE], dt_in,
                                   isOutput=False)
    # packed output: cols 0:384 bn stats, 384:400 hacc, 400:416 sqacc,
    # 416:418 hacc halves of piece 2, 418:420 sqacc halves of piece 2
    out = nc.declare_dram_parameter(
        "out", [P, NP_TOT * GPP * 6 + 2 * NP_TOT + 4], f32, isOutput=True)

    with tile.TileContext(nc) as tc:
        with (
            tc.tile_pool(name="pc", bufs=NP_TOT) as piece_pool,
            tc.tile_pool(name="acc", bufs=1) as acc_pool,
        ):
            nb = NP_TOT * GPP * 6
            outT = acc_pool.tile([P, nb + 2 * NP_TOT + 4], f32, tag="out")
            bnT = outT[:, 0:nb]
            haccT = outT[:, nb:nb + NP_TOT]
            sqaccT = outT[:, nb + NP_TOT:nb + 2 * NP_TOT]
            haccT2 = outT[:, nb + 2 * NP_TOT:nb + 2 * NP_TOT + 2]
            sqaccT2 = outT[:, nb + 2 * NP_TOT + 2:nb + 2 * NP_TOT + 4]
            cp_scr = acc_pool.tile([P, PIECE], mybir.dt.float16, tag="cps")
            sq_scr = acc_pool.tile([P, PIECE], mybir.dt.float16, tag="sqs")

            # zero the accumulators (only assigned slices get written);
            # runs before the first DMA lands, so it's free
            nc.vector.memset(bnT[:, :], 0.0)
            nc.scalar.memzero(outT[:, nb:])

            # the first piece consumed by each engine is DMA'd in small
            # slices so the first compute granule lands early (a whole
            # 256 KiB piece takes ~7us once all queues compete)
            for i, a in enumerate(ASSIGN):
                if i == 0:  # first V piece: one DMA + bn_stats per group
                    subs = []
                    for g in range(GPP):
                        ts = piece_pool.tile([P, G], dt_in)
                        nc.sync.dma_start(
                            ts[:], ht[i].rearrange("p (g x) -> p g x",
                                                   x=G)[:, g, :])
                        subs.append(ts)
                    for g in range(GPP):
                        nc.vector.bn_stats(
                            bnT[:, (i * GPP + g) * 6:(i * GPP + g + 1) * 6],
                            subs[g][:])
                    continue
                if i == 2:  # first A piece: two halves
                    halves = []
                    for hhalf in range(2):
                        ts = piece_pool.tile([P, PIECE // 2], dt_in)
                        nc.sync.dma_start(
                            ts[:], ht[i][:, hhalf * (PIECE // 2):
                                         (hhalf + 1) * (PIECE // 2)])
                        halves.append(ts)
                    for hhalf, ts in enumerate(halves):
                        nc.scalar.activation(
                            cp_scr[:, 0:PIECE // 2], ts[:], AF.Copy,
                            accum_out=haccT2[:, hhalf:hhalf + 1])
                        nc.scalar.activation(
                            sq_scr[:, 0:PIECE // 2], ts[:], AF.Square,
                            accum_out=sqaccT2[:, hhalf:hhalf + 1])
                    continue

                t = piece_pool.tile([P, PIECE], dt_in)
                nc.sync.dma_start(t[:], ht[i])

                if a == "V":
                    t3 = t[:].rearrange("p (g x) -> p g x", x=G)
                    for g in range(GPP):
                        nc.vector.bn_stats(
                            bnT[:, (i * GPP + g) * 6:(i * GPP + g + 1) * 6],
                            t3[:, g, :])
                elif a == "A":
                    nc.scalar.activation(
                        cp_scr[:], t[:], AF.Copy,
                        accum_out=haccT[:, i:i + 1])
                    nc.scalar.activation(
                        sq_scr[:], t[:], AF.Square,
                        accum_out=sqaccT[:, i:i + 1])
                else:  # "S": DVE takes groups 0-1, ACT takes rows 1024:
                    t3 = t[:].rearrange("p (g x) -> p g x", x=G)
                    for g in range(2):
                        nc.vector.bn_stats(
                            bnT[:, (i * GPP + g) * 6:(i * GPP + g + 1) * 6],
                            t3[:, g, :])
                    half = slice(PIECE // 2, PIECE)
                    nc.scalar.activation(
                        cp_scr[:, 0:PIECE // 2], t[:, half], AF.Copy,
                        accum_out=haccT[:, i:i + 1])
                    nc.scalar.activation(
                        sq_scr[:, 0:PIECE // 2], t[:, half], AF.Square,
                        accum_out=sqaccT[:, i:i + 1])

            nc.sync.dma_start(out[:], outT[:])
    nc.compile()
    return nc


def _get_nc():
    if "nc" not in _NC_CACHE:
        _NC_CACHE["nc"] = _build_nc()
    return _NC_CACHE["nc"]


def _piece_granules(i):
    """(row_offset_within_piece, length, source, aux_idx) for piece i."""
    a = ASSIGN[i]
    if i == 2:  # first A piece DMA'd/accumulated as two halves
        return [(0, PIECE // 2, "acc2", 0), (PIECE // 2, PIECE // 2, "acc2", 1)]
    if a == "V":
        return [(g * G, G, "bn", 0) for g in range(GPP)]
    if a == "A":
        return [(0, PIECE, "acc", 0)]
    return [(0, G, "bn", 0), (G, G, "bn", 0),
            (PIECE // 2, PIECE // 2, "acc", 0)]


def _prep_core(hs_k):
    """hs_k already globally sorted. Returns device input [16, 128, 2048]."""
    return np.ascontiguousarray(
        hs_k.reshape(NPIECE, PIECE, NBLK, P).transpose(2, 0, 3, 1)
        .astype(_np_in_dtype())
    ).reshape(NP_TOT, P, PIECE)


def _core_stats(hs_k, ids_k, dev, sums, sumsq):
    """Accumulate per-class stats for one core into sums/sumsq [C, D] f64.

    Pure-class granules use device stats; granules spanning a class
    boundary are recomputed exactly on the host from the raw f32 rows.
    """
    packed = dev["out"].astype(np.float64)
    nb = NP_TOT * GPP * 6
    bnr = packed[:, 0:nb].reshape(P, NP_TOT, GPP, 6)
    ha = packed[:, nb:nb + NP_TOT]
    sq = packed[:, nb + NP_TOT:nb + 2 * NP_TOT]
    ha2 = packed[:, nb + 2 * NP_TOT:nb + 2 * NP_TOT + 2]
    sq2 = packed[:, nb + 2 * NP_TOT + 2:nb + 2 * NP_TOT + 4]
    for i in range(NP_TOT):
        b, p = divmod(i, NPIECE)
        fsl = slice(b * P, (b + 1) * P)
        for off, ln, src, aux in _piece_granules(i):
            r0 = p * PIECE + off
            r1 = r0 + ln
            if ids_k[r0] == ids_k[r1 - 1]:
                c = int(ids_k[r0])
                if src == "bn":
                    ce, me, m2e, co, mo, m2o = bnr[:, i, off // G, :].T
                    sums[c, fsl] += ce * me + co * mo
                    sumsq[c, fsl] += m2e + ce * me * me + m2o + co * mo * mo
                elif src == "acc":
                    sums[c, fsl] += ha[:, i]
                    sumsq[c, fsl] += sq[:, i]
                else:
                    sums[c, fsl] += ha2[:, aux]
                    sumsq[c, fsl] += sq2[:, aux]
            else:
                rows = hs_k[r0:r1, fsl].astype(np.float64)
                rids = ids_k[r0:r1]
                for q in np.unique(rids):
                    sel = rows[rids == q]
                    sums[q, fsl] += sel.sum(axis=0)
                    sumsq[q, fsl] += (sel * sel).sum(axis=0)


def _device_stats(hidden, ids, **run_kwargs):
    """Returns (sums[C,D], sumsq[C,D]) float64, plus the raw run result."""
    from concourse import bass_utils

    nc = _get_nc()

    order = np.argsort(ids, kind="stable")       # GLOBAL sort by class
    ids_s = ids[order]
    hs = hidden[order]

    in_maps = []
    for k in range(NCORES):
        rows = slice(k * ROWS, (k + 1) * ROWS)
        in_maps.append({"ht": _prep_core(hs[rows])})

    res = bass_utils.run_bass_kernel_spmd(nc, in_maps, list(range(NCORES)),
                                          **run_kwargs)

    sums = np.zeros((C, D), dtype=np.float64)
    sumsq = np.zeros((C, D), dtype=np.float64)
    for k in range(NCORES):
        rows = slice(k * ROWS, (k + 1) * ROWS)
        _core_stats(hs[rows], ids_s[rows], res.results[k], sums, sumsq)
    return sums, sumsq, res


def _pairwise_loss(counts, sums, sumsq, d):
    """The tiny O(C^2 D) stage on host CPU.

    Runs in float32 with the same jax ops as the reference: at these extreme
    betainc parameters (b ~ 8190, x ~ 1e-5) jax's f32 betainc differs from
    the true (f64) value by ~1e-3, so matching the reference requires
    replicating its f32 numerics, not improving on them.
    """
    import jax
    import jax.numpy as jnp

    cpu = jax.devices("cpu")[0]
    with jax.default_device(cpu):
        counts64 = counts.astype(np.float64)
        means64 = sums / counts64[:, None]
        withins64 = sumsq - counts64[:, None] * means64**2
        counts = jnp.asarray(counts64, jnp.float32)               # [C]
        means = jnp.asarray(means64, jnp.float32)                 # [C, D]
        withins = jnp.asarray(withins64, jnp.float32)             # [C, D]
        half_diff = (means[:, None, :] - means[None, :, :]) * 0.5
        pair_counts = counts[:, None] + counts[None, :]
        pair_between = half_diff * half_diff * pair_counts[:, :, None]
        pair_within = withins[:, None, :] + withins[None, :, :]
        d2 = pair_counts - 2.0
        d2 = jnp.where(d2 == 0.0, 1e-5, d2)
        x = pair_between / (pair_between + pair_within)
        x = jnp.clip(x, XMIN, XMAX)
        a = jnp.full_like(x, 0.5)
        b = jnp.broadcast_to((d2 * 0.5)[:, :, None], x.shape)
        xbetainc = jax.scipy.special.betainc(a, b, x)             # [C, C, D]
        top_k, _ = jax.lax.top_k(xbetainc, int(d))                # [C, C, d]
        per_pair = jnp.sum(jnp.log(top_k), axis=-1)               # [C, C]
        mask = jnp.triu(jnp.ones((C, C), dtype=bool), k=1)
        total = jnp.sum(jnp.where(mask, per_pair, jnp.zeros_like(per_pair)))
        return float(-total)


def kernel(hidden, batch_ids, d):
    hidden = np.asarray(hidden, dtype=np.float32)
    ids = np.asarray(batch_ids).astype(np.int64)
    assert hidden.shape == (N, D), hidden.shape

    counts = np.bincount(ids, minlength=C).astype(np.float64)
    sums, sumsq, _ = _device_stats(hidden, ids)
    total = _pairwise_loss(counts, sums, sumsq, int(np.asarray(d)))
    return np.array(total, dtype=np.float32)


# revision 17
# speedup vs baseline: 1.0053x; 1.0053x over previous
"""Bass/Trainium2 kernel for nn_F_Loss_65446711656630.

Strategy (data-parallel over N, 8 cores, fp8 e4m3 inputs):
  - Host: GLOBAL stable sort of all rows by class id, slice 8192 rows/core.
  - Per core the rows are split between three engine pipelines so all three
    finish together:
      * rows 0:4096 ("A part"), transposed to 8 pieces of
        [128 features x 2048 rows]: DVE bn_stats in 512-row groups. One DVE
        pass yields count/mean/M2 -> BOTH the h-sum and the h^2-sum per
        granule.
      * rows 4096:8192 ("B part"), natural [row, feature] layout as 32
        chunks of [128 rows x 512 features]: ACT squares each chunk
        (fp8 -> fp16), and the PE contracts both the raw chunk and its
        square against per-chunk one-hot class matrices, accumulating
        exact per-class sums in PSUM. No class-boundary fixups needed for
        this half at all.
  - DMA descriptor generation is split between the Sync and Tensor queues
    (kicks serialize at ~0.6us each on one queue).
  - Host: per-class stats from pure-class bn granules (f64) + exact host
    recompute for granules spanning a class boundary + the PSUM partials;
    then the tiny O(C^2 D) pairwise betainc/top-k stage in f32 jax on CPU
    (mirroring the reference's numerics exactly).

  fp8 e4m3 input costs ~5e-4 final rel err (gate is 2e-2) and halves HBM
  traffic to 4 MiB/core.
"""

import numpy as np

C = 16
D = 512
N = 65536
NCORES = 8
ROWS = N // NCORES          # 8192 rows per core
P = 128                     # SBUF partitions
PIECE = 2048                # rows per layout-A piece
G = 512                     # bn_stats group size (HW max)
GPP = PIECE // G            # 4 groups per piece
NBLK = D // P               # 4 feature blocks
A_ROWS = 4096               # rows handled by bn_stats (layout A)
NPIECE_A = A_ROWS // PIECE  # 2 row-pieces
NP_A = NBLK * NPIECE_A      # 8 layout-A pieces
B_ROWS = ROWS - A_ROWS      # 4096 rows handled by PE (layout B)
NCHUNK = B_ROWS // P        # 32 chunks
NBG = 4                     # B DMA groups
CPG = NCHUNK // NBG         # 8 chunks per group
XMIN, XMAX = 1e-37, 1.0 - 1e-5

_NC_CACHE = {}


def _np_fp8():
    import ml_dtypes
    return ml_dtypes.float8_e4m3fn


def _build_nc():
    """Per-core SPMD program.

    Inputs:  "ht"   [8, 128, 2048] fp8   layout-A pieces (i = b*2 + p)
             "hb"   [4, 8, 128, 512] fp8 layout-B chunk groups
             "oh8"  [128, 512] fp8       one-hot, col c*16+m = row c*128+p in class m
             "oh16" [128, 512] fp16      same in fp16 (for the squared pass)
    Outputs: "outa" [128, 192] f32       bn stats (piece i group g -> cols (i*4+g)*6..+6)
             "outb" [16, 1024] f32       cols 0:512 per-class h sums,
                                         512:1024 per-class h^2 sums (B rows)
    """
    import concourse.tile as tile
    from concourse import bacc, mybir

    f32 = mybir.dt.float32
    f16 = mybir.dt.float16
    f8 = mybir.dt.float8e4
    AF = mybir.ActivationFunctionType

    nc = bacc.Bacc("TRN2", target_bir_lowering=False, debug=False,
                   num_devices=NCORES)
    ht = nc.declare_dram_parameter("ht", [NP_A, P, PIECE], f8, isOutput=False)
    hb = nc.declare_dram_parameter("hb", [NBG, CPG, P, D], f8, isOutput=False)
    oh8 = nc.declare_dram_parameter("oh8", [P, NCHUNK * C], f8, isOutput=False)
    oh16 = nc.declare_dram_parameter("oh16", [P, NCHUNK * C], f16,
                                     isOutput=False)
    outa = nc.declare_dram_parameter("outa", [P, NP_A * GPP * 6], f32,
                                     isOutput=True)
    outb = nc.declare_dram_parameter("outb", [C, 2 * D], f32, isOutput=True)

    with tile.TileContext(nc) as tc:
        with (
            tc.tile_pool(name="pa", bufs=NP_A + 3) as pa_pool,
            tc.tile_pool(name="pb", bufs=NBG) as pb_pool,
            tc.tile_pool(name="sq", bufs=6) as sq_pool,
            tc.tile_pool(name="acc", bufs=1) as acc_pool,
            tc.tile_pool(name="ps", bufs=2, space="PSUM") as psum_pool,
        ):
            bnT = acc_pool.tile([P, NP_A * GPP * 6], f32, tag="bn")
            evacT = acc_pool.tile([P, 2 * D], f32, tag="evac")
            oh8T = acc_pool.tile([P, NCHUNK * C], f8, tag="oh8")
            oh16T = acc_pool.tile([P, NCHUNK * C], f16, tag="oh16")
            psum_h = psum_pool.tile([C, D], f32, tag="ph")
            psum_sq = psum_pool.tile([C, D], f32, tag="pq")

            nc.vector.memset(bnT[:], 0.0)

            # ---- DMA kicks, interleaved for early starts.  Sync kicks the
            # first sub-pieces + one-hots + half the rest; Tensor (idle
            # until its first matmul) kicks the other half.
            a_tiles = {}
            b_tiles = {}
            # first A piece in 4 sub-slices so DVE starts early
            sub0 = []
            for g in range(GPP):
                ts_ = pa_pool.tile([P, G], f8)
                nc.sync.dma_start(
                    ts_[:], ht[0].rearrange("p (g x) -> p g x", x=G)[:, g, :])
                sub0.append(ts_)
            # first B group in two halves so ACT/PE start early
            bg0 = pb_pool.tile([P, CPG, D], f8)
            nc.sync.dma_start(bg0[:, 0:CPG // 2, :], hb[0][0:CPG // 2]
                              .rearrange("c p f -> p c f"))
            nc.sync.dma_start(oh8T[:], oh8[:])
            nc.sync.dma_start(oh16T[:], oh16[:])
            nc.sync.dma_start(bg0[:, CPG // 2:, :], hb[0][CPG // 2:]
                              .rearrange("c p f -> p c f"))
            b_tiles[0] = bg0
            # Scalar's queue is idle until B data lands (~9us); 3 kicks fit
            # in that dead time and halve the Sync kick serialization.
            for q in (1, 2, 3):
                bg = pb_pool.tile([P, CPG, D], f8)
                nc.scalar.dma_start(bg[:], hb[q].rearrange("c p f -> p c f"))
                b_tiles[q] = bg
            for i in range(1, NP_A):
                t = pa_pool.tile([P, PIECE], f8)
                nc.sync.dma_start(t[:], ht[i])
                a_tiles[i] = t

            # ---- DVE: bn_stats over all layout-A pieces
            for g in range(GPP):
                nc.vector.bn_stats(bnT[:, g * 6:(g + 1) * 6], sub0[g][:])
            for i in range(1, NP_A):
                t3 = a_tiles[i][:].rearrange("p (g x) -> p g x", x=G)
                for g in range(GPP):
                    j = i * GPP + g
                    nc.vector.bn_stats(bnT[:, j * 6:(j + 1) * 6], t3[:, g, :])

            # ---- ACT squares + PE one-hot contractions over B chunks
            for c in range(NCHUNK):
                q, j = divmod(c, CPG)
                mov = b_tiles[q][:, j, :]
                nc.tensor.matmul(
                    psum_h[:], oh8T[:, c * C:(c + 1) * C], mov,
                    start=(c == 0), stop=(c == NCHUNK - 1))
                sq = sq_pool.tile([P, D], f16)
                nc.scalar.activation(sq[:], mov, AF.Square)
                nc.tensor.matmul(
                    psum_sq[:], oh16T[:, c * C:(c + 1) * C], sq[:],
                    start=(c == 0), stop=(c == NCHUNK - 1))

            # ---- evacuate PSUM (DVE + ACT in parallel), then DMA out
            nc.sync.dma_start(outa[:], bnT[:])
            nc.vector.tensor_copy(evacT[0:C, 0:D], psum_h[:])
            nc.scalar.copy(evacT[0:C, D:2 * D], psum_sq[:])
            nc.sync.dma_start(outb[:], evacT[0:C, :])
    nc.compile()
    return nc


def _get_nc():
    if "nc" not in _NC_CACHE:
        _NC_CACHE["nc"] = _build_nc()
    return _NC_CACHE["nc"]


def _prep_core(hs_k, ids_k):
    """hs_k/ids_k globally sorted; returns the device input map."""
    fp8 = _np_fp8()
    ha = hs_k[0:A_ROWS]
    T = np.ascontiguousarray(
        ha.reshape(NPIECE_A, PIECE, NBLK, P).transpose(2, 0, 3, 1)
        .astype(fp8)
    ).reshape(NP_A, P, PIECE)
    hbm = np.ascontiguousarray(
        hs_k[A_ROWS:].astype(fp8).reshape(NBG, CPG, P, D))
    ids_b = np.asarray(ids_k[A_ROWS:]).reshape(NCHUNK, P)
    # oh[p, c*16+m] = 1 iff row c*128+p belongs to class m
    oh = (ids_b[:, :, None] == np.arange(C)[None, None, :])
    oh = np.ascontiguousarray(oh.transpose(1, 0, 2).reshape(P, NCHUNK * C))
    return {
        "ht": T,
        "hb": hbm,
        "oh8": oh.astype(fp8),
        "oh16": oh.astype(np.float16),
    }


def _core_stats(hs_k, ids_k, dev, sums, sumsq):
    """Accumulate per-class stats for one core into sums/sumsq [C, D] f64."""
    bnr = dev["outa"].astype(np.float64).reshape(P, NP_A, GPP, 6)
    for i in range(NP_A):
        b, p = divmod(i, NPIECE_A)
        fsl = slice(b * P, (b + 1) * P)
        for g in range(GPP):
            r0 = p * PIECE + g * G
            r1 = r0 + G
            if ids_k[r0] == ids_k[r1 - 1]:
                cc = int(ids_k[r0])
                ce, me, m2e, co, mo, m2o = bnr[:, i, g, :].T
                sums[cc, fsl] += ce * me + co * mo
                sumsq[cc, fsl] += m2e + ce * me * me + m2o + co * mo * mo
            else:
                rows = hs_k[r0:r1, fsl].astype(np.float64)
                rids = ids_k[r0:r1]
                for q in np.unique(rids):
                    sel = rows[rids == q]
                    sums[q, fsl] += sel.sum(axis=0)
                    sumsq[q, fsl] += (sel * sel).sum(axis=0)
    outb = dev["outb"].astype(np.float64)
    sums += outb[:, 0:D]
    sumsq += outb[:, D:2 * D]


def _device_stats(hidden, ids, **run_kwargs):
    """Returns (sums[C,D], sumsq[C,D]) float64, plus the raw run result."""
    from concourse import bass_utils

    nc = _get_nc()

    order = np.argsort(ids, kind="stable")       # GLOBAL sort by class
    ids_s = ids[order]
    hs = hidden[order]

    in_maps = []
    for k in range(NCORES):
        rows = slice(k * ROWS, (k + 1) * ROWS)
        in_maps.append(_prep_core(hs[rows], ids_s[rows]))

    res = bass_utils.run_bass_kernel_spmd(nc, in_maps, list(range(NCORES)),
                                          **run_kwargs)

    sums = np.zeros((C, D), dtype=np.float64)
    sumsq = np.zeros((C, D), dtype=np.float64)
    for k in range(NCORES):
        rows = slice(k * ROWS, (k + 1) * ROWS)
        _core_stats(hs[rows], ids_s[rows], res.results[k], sums, sumsq)
    return sums, sumsq, res


def _pairwise_loss(counts, sums, sumsq, d):
    """The tiny O(C^2 D) stage on host CPU.

    Runs in float32 with the same jax ops as the reference: at these extreme
    betainc parameters (b ~ 8190, x ~ 1e-5) jax's f32 betainc differs from
    the true (f64) value by ~1e-3, so matching the reference requires
    replicating its f32 numerics, not improving on them.
    """
    import jax
    import jax.numpy as jnp

    cpu = jax.devices("cpu")[0]
    with jax.default_device(cpu):
        counts64 = counts.astype(np.float64)
        means64 = sums / counts64[:, None]
        withins64 = sumsq - counts64[:, None] * means64**2
        counts = jnp.asarray(counts64, jnp.float32)               # [C]
        means = jnp.asarray(means64, jnp.float32)                 # [C, D]
        withins = jnp.asarray(withins64, jnp.float32)             # [C, D]
        half_diff = (means[:, None, :] - means[None, :, :]) * 0.5
        pair_counts = counts[:, None] + counts[None, :]
        pair_between = half_diff * half_diff * pair_counts[:, :, None]
        pair_within = withins[:, None, :] + withins[None, :, :]
        d2 = pair_counts - 2.0
        d2 = jnp.where(d2 == 0.0, 1e-5, d2)
        x = pair_between / (pair_between + pair_within)
        x = jnp.clip(x, XMIN, XMAX)
        a = jnp.full_like(x, 0.5)
        b = jnp.broadcast_to((d2 * 0.5)[:, :, None], x.shape)
        xbetainc = jax.scipy.special.betainc(a, b, x)             # [C, C, D]
        top_k, _ = jax.lax.top_k(xbetainc, int(d))                # [C, C, d]
        per_pair = jnp.sum(jnp.log(top_k), axis=-1)               # [C, C]
        mask = jnp.triu(jnp.ones((C, C), dtype=bool), k=1)
        total = jnp.sum(jnp.where(mask, per_pair, jnp.zeros_like(per_pair)))
        return float(-total)


def kernel(hidden, batch_ids, d):
    hidden = np.asarray(hidden, dtype=np.float32)
    ids = np.asarray(batch_ids).astype(np.int64)
    assert hidden.shape == (N, D), hidden.shape

    counts = np.bincount(ids, minlength=C).astype(np.float64)
    sums, sumsq, _ = _device_stats(hidden, ids)
    total = _pairwise_loss(counts, sums, sumsq, int(np.asarray(d)))
    return np.array(total, dtype=np.float32)


# revision 19
# speedup vs baseline: 1.0784x; 1.0727x over previous
"""Bass/Trainium2 kernel for nn_F_Loss_65446711656630.

Strategy (data-parallel over N, 8 cores, fp8 e4m3 inputs):
  - Host: GLOBAL stable sort of all rows by class id, slice 8192 rows/core.
  - Per core the rows are split between three engine pipelines so all three
    finish together:
      * rows 0:4608 ("A part"), transposed to [128 features x rows] pieces:
        DVE bn_stats in 512-row groups. One DVE pass yields count/mean/M2
        -> BOTH the h-sum and the h^2-sum per granule.
      * rows 4608:8192 ("B part"), natural [row, feature] layout as 28
        chunks of [128 rows x 512 features]: ACT squares each chunk
        (fp8 -> fp16), and the PE contracts the raw chunks (fp8 DoubleRow,
        two chunks per matmul) and the squares (fp16) against per-chunk
        one-hot class matrices, accumulating exact per-class sums in PSUM.
        No class-boundary fixups needed for this half.
  - Host: per-class stats from pure-class bn granules (f64) + exact host
    recompute for granules spanning a class boundary + the PSUM partials;
    then the tiny O(C^2 D) pairwise betainc/top-k stage in f32 jax on CPU
    (mirroring the reference's numerics exactly).

  fp8 e4m3 input costs ~6e-4 final rel err (gate is 2e-2) and halves HBM
  traffic to ~4 MiB/core.  Squares stay fp16: fp8 squares would cost
  ~6e-3.
"""

import numpy as np

C = 16
D = 512
N = 65536
NCORES = 8
ROWS = N // NCORES          # 8192 rows per core
P = 128                     # SBUF partitions
PIECE = 2048                # rows per full layout-A piece
G = 512                     # bn_stats group size (HW max)
GPP = PIECE // G            # 4 groups per piece
NBLK = D // P               # 4 feature blocks
A_ROWS = 4608               # rows handled by bn_stats (layout A)
NPIECE_A = 2                # full 2048-row pieces per block (rows 0:4096)
NP_A = NBLK * NPIECE_A      # 8 full layout-A pieces
Q_ROWS = A_ROWS - NPIECE_A * PIECE   # 512 rows in the quarter pieces
B_ROWS = ROWS - A_ROWS      # 3584 rows handled by PE (layout B)
NCHUNK = B_ROWS // P        # 28 chunks
NPAIR = NCHUNK // 2         # 14 DoubleRow pairs
BGROUPS = [(0, 4), (4, 8), (8, 16), (16, 24), (24, 28)]  # chunk ranges per DMA
NBN = NP_A * GPP + NBLK     # 36 bn_stats granules
XMIN, XMAX = 1e-37, 1.0 - 1e-5

_NC_CACHE = {}


def _np_fp8():
    import ml_dtypes
    return ml_dtypes.float8_e4m3fn


def _build_nc():
    """Per-core SPMD program.

    Inputs:  "ht"   [8, 128, 2048] fp8   layout-A pieces (i = b*2 + p)
             "ht2"  [4, 128, 512] fp8    layout-A quarter pieces (block b,
                                         rows 4096:4608)
             "hb"   [28, 128, 512] fp8   layout-B chunks
             "oh8"  [128, 448] fp8       one-hot, col c*16+m <-> row c*128+p
             "oh16" [128, 448] fp16      same in fp16 (for the squared pass)
    Outputs: "outa" [128, 216] f32       bn stats, granule j -> cols j*6..+6
             "outb" [16, 1024] f32       cols 0:512 per-class h sums,
                                         512:1024 per-class h^2 sums (B rows)
    """
    import concourse.tile as tile
    from concourse import bacc, mybir

    f32 = mybir.dt.float32
    f16 = mybir.dt.float16
    f8 = mybir.dt.float8e4
    AF = mybir.ActivationFunctionType

    nc = bacc.Bacc("TRN2", target_bir_lowering=False, debug=False,
                   num_devices=NCORES)
    ht = nc.declare_dram_parameter("ht", [NP_A, P, PIECE], f8, isOutput=False)
    ht2 = nc.declare_dram_parameter("ht2", [NBLK, P, Q_ROWS], f8,
                                    isOutput=False)
    hb = nc.declare_dram_parameter("hb", [NCHUNK, P, D], f8, isOutput=False)
    oh8 = nc.declare_dram_parameter("oh8", [P, NCHUNK * C], f8, isOutput=False)
    oh16 = nc.declare_dram_parameter("oh16", [P, NCHUNK * C], f16,
                                     isOutput=False)
    outa = nc.declare_dram_parameter("outa", [P, NBN * 6], f32, isOutput=True)
    outb = nc.declare_dram_parameter("outb", [C, 2 * D], f32, isOutput=True)

    with tile.TileContext(nc) as tc:
        with (
            tc.tile_pool(name="pa", bufs=NP_A + 6) as pa_pool,
            tc.tile_pool(name="pb", bufs=len(BGROUPS)) as pb_pool,
            tc.tile_pool(name="sq", bufs=8) as sq_pool,
            tc.tile_pool(name="acc", bufs=1) as acc_pool,
            tc.tile_pool(name="ps", bufs=2, space="PSUM") as psum_pool,
        ):
            bnT = acc_pool.tile([P, NBN * 6], f32, tag="bn")
            evacT = acc_pool.tile([P, 2 * D], f32, tag="evac")
            oh8T = acc_pool.tile([P, NCHUNK * C], f8, tag="oh8")
            oh16T = acc_pool.tile([P, NCHUNK * C], f16, tag="oh16")
            psum_h = psum_pool.tile([C, D], f32, tag="ph")
            psum_sq = psum_pool.tile([C, D], f32, tag="pq")

            nc.vector.memset(bnT[:], 0.0)

            # ---- DMA kicks on Sync, ordered to match consumption so each
            # engine starts as early as possible (kicks serialize ~0.6us).
            bg = {}

            def kick_b(gi):
                c0, c1 = BGROUPS[gi]
                t = pb_pool.tile([P, c1 - c0, D], f8)
                nc.sync.dma_start(t[:], hb[c0:c1].rearrange("c p f -> p c f"))
                bg[gi] = t

            s0a = pa_pool.tile([P, G], f8)
            nc.sync.dma_start(
                s0a[:], ht[0].rearrange("p (g x) -> p g x", x=G)[:, 0, :])
            kick_b(0)
            s0b = pa_pool.tile([P, PIECE - G], f8)
            nc.sync.dma_start(s0b[:], ht[0][:, G:])
            a_tiles = {}

            def kick_a(i):
                t = pa_pool.tile([P, PIECE], f8)
                nc.sync.dma_start(t[:], ht[i])
                a_tiles[i] = t

            kick_a(1)
            nc.sync.dma_start(oh8T[:], oh8[:])
            kick_b(1)
            nc.sync.dma_start(oh16T[:], oh16[:])
            kick_a(2)
            kick_b(2)
            kick_a(3)
            kick_a(4)
            kick_b(3)
            kick_a(5)
            kick_b(4)
            kick_a(6)
            kick_a(7)
            qt = pa_pool.tile([P, NBLK, Q_ROWS], f8)
            nc.sync.dma_start(qt[:], ht2[:].rearrange("b p x -> p b x"))

            # ---- DVE: bn_stats over all layout-A granules
            nc.vector.bn_stats(bnT[:, 0:6], s0a[:])
            s0b3 = s0b[:].rearrange("p (g x) -> p g x", x=G)
            for g in range(GPP - 1):
                j = 1 + g
                nc.vector.bn_stats(bnT[:, j * 6:(j + 1) * 6], s0b3[:, g, :])
            for i in range(1, NP_A):
                t3 = a_tiles[i][:].rearrange("p (g x) -> p g x", x=G)
                for g in range(GPP):
                    j = i * GPP + g
                    nc.vector.bn_stats(bnT[:, j * 6:(j + 1) * 6], t3[:, g, :])
            for b in range(NBLK):
                j = NP_A * GPP + b
                nc.vector.bn_stats(bnT[:, j * 6:(j + 1) * 6], qt[:, b, :])

            # ---- ACT squares + PE contractions over B chunks
            def chunk_ap(c):
                for gi, (c0, c1) in enumerate(BGROUPS):
                    if c0 <= c < c1:
                        return bg[gi][:, c - c0, :]

            def pair_ap(j):
                c = 2 * j
                for gi, (c0, c1) in enumerate(BGROUPS):
                    if c0 <= c < c1:
                        assert c + 1 < c1
                        return bg[gi][:, c - c0:c - c0 + 2, :]

            for j in range(NPAIR):
                c0, c1 = 2 * j, 2 * j + 1
                nc.tensor.matmul(
                    psum_h[:], oh8T[:, c0 * C:(c1 + 1) * C]
                    .rearrange("p (c m) -> p c m", m=C),
                    pair_ap(j),
                    start=(j == 0), stop=(j == NPAIR - 1),
                    perf_mode=mybir.MatmulPerfMode.DoubleRow)
                for c in (c0, c1):
                    sq = sq_pool.tile([P, D], f16)
                    nc.scalar.activation(sq[:], chunk_ap(c), AF.Square)
                    nc.tensor.matmul(
                        psum_sq[:], oh16T[:, c * C:(c + 1) * C], sq[:],
                        start=(c == 0), stop=(c == NCHUNK - 1))

            # ---- evacuate PSUM (DVE + ACT in parallel), then DMA out
            nc.sync.dma_start(outa[:], bnT[:])
            nc.vector.tensor_copy(evacT[0:C, 0:D], psum_h[:])
            nc.scalar.copy(evacT[0:C, D:2 * D], psum_sq[:])
            nc.sync.dma_start(outb[:], evacT[0:C, :])
    nc.compile()
    return nc


def _get_nc():
    if "nc" not in _NC_CACHE:
        _NC_CACHE["nc"] = _build_nc()
    return _NC_CACHE["nc"]


def _prep_core(hs_k, ids_k):
    """hs_k/ids_k globally sorted; returns the device input map."""
    fp8 = _np_fp8()
    T = np.ascontiguousarray(
        hs_k[0:NPIECE_A * PIECE]
        .reshape(NPIECE_A, PIECE, NBLK, P).transpose(2, 0, 3, 1)
        .astype(fp8)
    ).reshape(NP_A, P, PIECE)
    T2 = np.ascontiguousarray(
        hs_k[NPIECE_A * PIECE:A_ROWS]
        .reshape(Q_ROWS, NBLK, P).transpose(1, 2, 0).astype(fp8))
    hbm = np.ascontiguousarray(
        hs_k[A_ROWS:].astype(fp8).reshape(NCHUNK, P, D))
    ids_b = np.asarray(ids_k[A_ROWS:]).reshape(NCHUNK, P)
    # oh[p, c*16+m] = 1 iff row c*128+p belongs to class m
    oh = (ids_b[:, :, None] == np.arange(C)[None, None, :])
    oh = np.ascontiguousarray(oh.transpose(1, 0, 2).reshape(P, NCHUNK * C))
    return {
        "ht": T,
        "ht2": T2,
        "hb": hbm,
        "oh8": oh.astype(fp8),
        "oh16": oh.astype(np.float16),
    }


def _granules():
    """Yields (bn_col_index, feature_block, row0) for every bn granule."""
    for i in range(NP_A):
        b, p = divmod(i, NPIECE_A)
        for g in range(GPP):
            yield i * GPP + g, b, p * PIECE + g * G
    for b in range(NBLK):
        yield NP_A * GPP + b, b, NPIECE_A * PIECE


def _core_stats(hs_k, ids_k, dev, sums, sumsq):
    """Accumulate per-class stats for one core into sums/sumsq [C, D] f64."""
    bnr = dev["outa"].astype(np.float64).reshape(P, NBN, 6)
    for j, b, r0 in _granules():
        fsl = slice(b * P, (b + 1) * P)
        r1 = r0 + G
        if ids_k[r0] == ids_k[r1 - 1]:
            cc = int(ids_k[r0])
            ce, me, m2e, co, mo, m2o = bnr[:, j, :].T
            sums[cc, fsl] += ce * me + co * mo
            sumsq[cc, fsl] += m2e + ce * me * me + m2o + co * mo * mo
        else:
            rows = hs_k[r0:r1, fsl].astype(np.float64)
            rids = ids_k[r0:r1]
            for q in np.unique(rids):
                sel = rows[rids == q]
                sums[q, fsl] += sel.sum(axis=0)
                sumsq[q, fsl] += (sel * sel).sum(axis=0)
    outb = dev["outb"].astype(np.float64)
    sums += outb[:, 0:D]
    sumsq += outb[:, D:2 * D]


def _device_stats(hidden, ids, **run_kwargs):
    """Returns (sums[C,D], sumsq[C,D]) float64, plus the raw run result."""
    from concourse import bass_utils

    nc = _get_nc()

    order = np.argsort(ids, kind="stable")       # GLOBAL sort by class
    ids_s = ids[order]
    hs = hidden[order]

    in_maps = []
    for k in range(NCORES):
        rows = slice(k * ROWS, (k + 1) * ROWS)
        in_maps.append(_prep_core(hs[rows], ids_s[rows]))

    res = bass_utils.run_bass_kernel_spmd(nc, in_maps, list(range(NCORES)),
                                          **run_kwargs)

    sums = np.zeros((C, D), dtype=np.float64)
    sumsq = np.zeros((C, D), dtype=np.float64)
    for k in range(NCORES):
        rows = slice(k * ROWS, (k + 1) * ROWS)
        _core_stats(hs[rows], ids_s[rows], res.results[k], sums, sumsq)
    return sums, sumsq, res


def _pairwise_loss(counts, sums, sumsq, d):
    """The tiny O(C^2 D) stage on host CPU.

    Runs in float32 with the same jax ops as the reference: at these extreme
    betainc parameters (b ~ 8190, x ~ 1e-5) jax's f32 betainc differs from
    the true (f64) value by ~1e-3, so matching the reference requires
    replicating its f32 numerics, not improving on them.
    """
    import jax
    import jax.numpy as jnp

    cpu = jax.devices("cpu")[0]
    with jax.default_device(cpu):
        counts64 = counts.astype(np.float64)
        means64 = sums / counts64[:, None]
        withins64 = sumsq - counts64[:, None] * means64**2
        counts = jnp.asarray(counts64, jnp.float32)               # [C]
        means = jnp.asarray(means64, jnp.float32)                 # [C, D]
        withins = jnp.asarray(withins64, jnp.float32)             # [C, D]
        half_diff = (means[:, None, :] - means[None, :, :]) * 0.5
        pair_counts = counts[:, None] + counts[None, :]
        pair_between = half_diff * half_diff * pair_counts[:, :, None]
        pair_within = withins[:, None, :] + withins[None, :, :]
        d2 = pair_counts - 2.0
        d2 = jnp.where(d2 == 0.0, 1e-5, d2)
        x = pair_between / (pair_between + pair_within)
        x = jnp.clip(x, XMIN, XMAX)
        a = jnp.full_like(x, 0.5)
        b = jnp.broadcast_to((d2 * 0.5)[:, :, None], x.shape)
        xbetainc = jax.scipy.special.betainc(a, b, x)             # [C, C, D]
        top_k, _ = jax.lax.top_k(xbetainc, int(d))                # [C, C, d]
        per_pair = jnp.sum(jnp.log(top_k), axis=-1)               # [C, C]
        mask = jnp.triu(jnp.ones((C, C), dtype=bool), k=1)
        total = jnp.sum(jnp.where(mask, per_pair, jnp.zeros_like(per_pair)))
        return float(-total)


def kernel(hidden, batch_ids, d):
    hidden = np.asarray(hidden, dtype=np.float32)
    ids = np.asarray(batch_ids).astype(np.int64)
    assert hidden.shape == (N, D), hidden.shape

    counts = np.bincount(ids, minlength=C).astype(np.float64)
    sums, sumsq, _ = _device_stats(hidden, ids)
    total = _pairwise_loss(counts, sums, sumsq, int(np.asarray(d)))
    return np.array(total, dtype=np.float32)
